# revision 1
# baseline (speedup 1.0000x reference)
"""Trainium2 Bass kernel for nn_MCLoss (scatter_memory forward).

Computes logits = inputs @ memory.T  ([4096, 2048] @ [2048, 50000] -> [4096, 50000] f32).

Strategy (tensor-parallel, per sharding hint): the memory bank is sharded
row-wise across 8 NeuronCores (6250 identity rows each, zero-padded to 6272 =
49*128 columns of the per-core logits slice). Each core computes its
[4096, 6272] slice of the logits with a tiled PE matmul; the host concatenates
the 8 slices and drops the padding.

Device kernel (per core, identical SPMD program):
  - lhs (stationary operand tiles): inputs pre-transposed on host into
    [128, 32, 16, 128] tile layout, dtype float32r.  A[p, m, k, j] =
    inputs[m*128 + j, k*128 + p], so lhsT tile (k, m) = A[:, m, k, :] is a
    [K=128, M=128] tile with the contraction dim on partitions.
  - rhs (moving operand): memory shard transposed on host to [2048, 6272]
    (memT[d, c] = memory[c, d]), dtype float32r.
  - float32r runs the 128x128 PE at 1 cycle/row for moving dims >= 256
    (bf16-class throughput) while keeping ~12-13 mantissa bits => rel err
    ~1.5e-4 on unit-norm rows, 16x better than bf16 at the same speed.
  - Loop nest: 5 column groups of width 1280 (paired with 3 PSUM banks,
    double-buffered), rhs group tiles resident in SBUF and reused by all 32
    m-tiles; per (group, m): one 1 MB lhs DMA, then 16 k-tiles x 3 bank
    slices of accumulating matmuls; PSUM evicted via VectorE copy to SBUF
    and DMA'd straight into the final [4096, 6272] layout.
"""
import numpy as np

import concourse.bass as bass
import concourse.mybir as mybir
import concourse.tile as tile
from concourse import bacc
from concourse.bass_utils import run_bass_kernel_spmd

P = 128
B = 4096          # rows of inputs
D = 2048          # features (contraction)
C = 50000         # memory rows (classes)
N_CORES = 8
N_SHARD = C // N_CORES          # 6250
N_PAD = 6272                    # 49 * 128, per-core padded logits width
CW = 1280                       # column-group width (3 PSUM banks: 512+512+256)
MT = B // P                     # 32
KT = D // P                     # 16

_NC_CACHE = {}


def _bank_slices(w):
    out, c = [], 0
    while c < w:
        s = min(512, w - c)
        out.append((c, s))
        c += s
    return out


def _build():
    if "nc" in _NC_CACHE:
        return _NC_CACHE["nc"]
    dt_in = mybir.dt.float32r
    nc = bacc.Bacc("TRN2", target_bir_lowering=False, debug=False)
    lhs = nc.dram_tensor("lhs", [P, MT, KT, P], dt_in, kind="ExternalInput")
    rhs = nc.dram_tensor("rhs", [D, N_PAD], dt_in, kind="ExternalInput")
    out = nc.dram_tensor("out", [B, N_PAD], mybir.dt.float32, kind="ExternalOutput")
    rhs_r = rhs[:].rearrange("(k p) c -> p k c", p=P)

    groups, c0 = [], 0
    while c0 < N_PAD:
        w = min(CW, N_PAD - c0)
        groups.append((c0, w))
        c0 += w

    with tile.TileContext(nc) as tc:
        with (
            tc.tile_pool(name="rhsp", bufs=2) as rhsp,
            tc.tile_pool(name="lhsp", bufs=4) as lhsp,
            tc.tile_pool(name="outp", bufs=2) as outp,
            tc.tile_pool(name="psump", bufs=2, space="PSUM") as psump,
        ):
            for c0, w in groups:
                rt = rhsp.tile([P, KT, w], dt_in, tag="rhs")
                nc.sync.dma_start(out=rt[:], in_=rhs_r[:, :, c0 : c0 + w])
                for m in range(MT):
                    lt = lhsp.tile([P, KT, P], dt_in, tag="lhs")
                    nc.sync.dma_start(out=lt[:], in_=lhs[:, m, :, :])
                    ps = psump.tile([P, w], mybir.dt.float32, tag="ps")
                    for k in range(KT):
                        for n0, nw in _bank_slices(w):
                            nc.tensor.matmul(
                                ps[:, n0 : n0 + nw],
                                lhsT=lt[:, k, :],
                                rhs=rt[:, k, n0 : n0 + nw],
                                start=(k == 0),
                                stop=(k == KT - 1),
                            )
                    ot = outp.tile([P, w], mybir.dt.float32, tag="out")
                    nc.vector.tensor_copy(out=ot[:], in_=ps[:])
                    nc.sync.dma_start(
                        out=out[m * P : (m + 1) * P, c0 : c0 + w], in_=ot[:]
                    )
    nc.compile()
    _NC_CACHE["nc"] = nc
    return nc


def _prep_inputs(inputs, memory):
    inputs = np.ascontiguousarray(np.asarray(inputs, dtype=np.float32))
    memory = np.asarray(memory, dtype=np.float32)
    # lhs tile layout: A[p, m, k, j] = inputs[m*128 + j, k*128 + p]
    lhs_np = np.ascontiguousarray(
        inputs.reshape(MT, P, KT, P).transpose(3, 0, 2, 1)
    )
    # per-core rhs: memT shard [D, N_PAD] with zero padding
    rhs_all = np.zeros((N_CORES, D, N_PAD), np.float32)
    rhs_all[:, :, :N_SHARD] = memory.reshape(N_CORES, N_SHARD, D).transpose(0, 2, 1)
    return lhs_np, rhs_all


def kernel(inputs, targets, memory):
    """Full-input entry point: returns logits [4096, 50000] float32."""
    nc = _build()
    lhs_np, rhs_all = _prep_inputs(inputs, memory)
    in_maps = [{"lhs": lhs_np, "rhs": rhs_all[c]} for c in range(N_CORES)]
    res = run_bass_kernel_spmd(nc, in_maps, core_ids=list(range(N_CORES)))
    logits = np.concatenate(
        [res.results[c]["out"][:, :N_SHARD] for c in range(N_CORES)], axis=1
    )
    return np.ascontiguousarray(logits)



# revision 2
# speedup vs baseline: 1.2008x; 1.2008x over previous
"""Trainium2 Bass kernel for nn_MCLoss (scatter_memory forward).

Computes logits = inputs @ memory.T  ([4096, 2048] @ [2048, 50000] -> [4096, 50000] f32).

Strategy (tensor-parallel, per sharding hint): the memory bank is sharded
row-wise across 8 NeuronCores (6250 identity rows each, zero-padded to 6272 =
49*128 columns of the per-core logits slice). Each core computes its
[4096, 6272] slice of the logits with a tiled PE matmul; the host concatenates
the 8 slices and drops the padding.

Device kernel (per core, identical SPMD program), v2:
  - All operands bf16 (PE runs 1 cycle/row, same as fp32r, but half the DMA
    bytes and half the SBUF footprint; rel err ~2e-3 on unit-norm rows, well
    inside the 2e-2 gate).
  - The whole lhs (inputs, pre-transposed on host into [128, 32, 16, 128]
    tile layout: A[p, m, k, j] = inputs[m*128 + j, k*128 + p]) stays RESIDENT
    in SBUF (16 MB = 128 KB/partition), loaded once at kernel start — v1
    re-streamed it once per column group (5x32 MB of DMA), which starved the
    PE. First 4 m-tiles are prefetched before the main loop; the rest stream
    in behind the first column group's compute.
  - rhs (memory shard transposed on host to [2048, 6272]) streams through
    SBUF in 12x512 + 1x128 column groups (one PSUM bank wide), triple
    buffered; each group is reused by all 32 m-tiles.
  - Per (group, m): 16 accumulating matmuls (one per k-tile) into one PSUM
    bank; VectorE evicts PSUM fp32 -> SBUF bf16 (downcast), DMA to the final
    [4096, 6272] bf16 layout; host upcasts to fp32.
  - PE work: 32*16*6272 = 3.21M cycles @ 2.4 GHz ~= 1.34 ms/core; total DMA
    ~93 MB/core (~70 GB/s), far under the ~400 GB/s fabric, so the kernel
    should sit at the PE roofline.
"""
import numpy as np
import ml_dtypes

import concourse.bass as bass
import concourse.mybir as mybir
import concourse.tile as tile
from concourse import bacc
from concourse.bass_utils import run_bass_kernel_spmd

P = 128
B = 4096          # rows of inputs
D = 2048          # features (contraction)
C = 50000         # memory rows (classes)
N_CORES = 8
N_SHARD = C // N_CORES          # 6250
N_PAD = 6272                    # 49 * 128, per-core padded logits width
GW = 512                        # column-group width (one PSUM bank of fp32)
MT = B // P                     # 32
KT = D // P                     # 16
LHS_PREFETCH = 4                # m-tiles loaded before the main loop

_NC_CACHE = {}


def _groups():
    out, c0 = [], 0
    while c0 < N_PAD:
        w = min(GW, N_PAD - c0)
        out.append((c0, w))
        c0 += w
    return out


def _build():
    if "nc" in _NC_CACHE:
        return _NC_CACHE["nc"]
    dt_in = mybir.dt.bfloat16
    nc = bacc.Bacc("TRN2", target_bir_lowering=False, debug=False)
    lhs = nc.dram_tensor("lhs", [P, MT, KT, P], dt_in, kind="ExternalInput")
    rhs = nc.dram_tensor("rhs", [D, N_PAD], dt_in, kind="ExternalInput")
    out = nc.dram_tensor("out", [B, N_PAD], dt_in, kind="ExternalOutput")
    rhs_r = rhs[:].rearrange("(k p) c -> p k c", p=P)

    with tile.TileContext(nc) as tc:
        with (
            tc.tile_pool(name="rhsp", bufs=3) as rhsp,
            tc.tile_pool(name="lhsp", bufs=1) as lhsp,
            tc.tile_pool(name="outp", bufs=4) as outp,
            tc.tile_pool(name="psump", bufs=4, space="PSUM") as psump,
        ):
            lhs_tiles = [None] * MT

            def _load_lhs(m):
                lt = lhsp.tile([P, KT, P], dt_in, tag=f"lhs{m}")
                nc.sync.dma_start(out=lt[:], in_=lhs[:, m, :, :])
                lhs_tiles[m] = lt

            for gi, (c0, w) in enumerate(_groups()):
                rt = rhsp.tile([P, KT, w], dt_in, tag="rhs")
                nc.sync.dma_start(out=rt[:], in_=rhs_r[:, :, c0 : c0 + w])
                if gi == 0:
                    for m in range(LHS_PREFETCH):
                        _load_lhs(m)
                for m in range(MT):
                    if gi == 0 and m + LHS_PREFETCH < MT:
                        _load_lhs(m + LHS_PREFETCH)
                    ps = psump.tile([P, w], mybir.dt.float32, tag="ps")
                    for k in range(KT):
                        nc.tensor.matmul(
                            ps[:],
                            lhsT=lhs_tiles[m][:, k, :],
                            rhs=rt[:, k, :],
                            start=(k == 0),
                            stop=(k == KT - 1),
                        )
                    ot = outp.tile([P, w], dt_in, tag="out")
                    nc.vector.tensor_copy(out=ot[:], in_=ps[:])
                    nc.sync.dma_start(
                        out=out[m * P : (m + 1) * P, c0 : c0 + w], in_=ot[:]
                    )
    nc.compile()
    _NC_CACHE["nc"] = nc
    return nc


def _prep_inputs(inputs, memory):
    bf16 = ml_dtypes.bfloat16
    inputs = np.ascontiguousarray(np.asarray(inputs, dtype=np.float32))
    memory = np.asarray(memory, dtype=np.float32)
    # lhs tile layout: A[p, m, k, j] = inputs[m*128 + j, k*128 + p]
    lhs_np = np.ascontiguousarray(
        inputs.reshape(MT, P, KT, P).transpose(3, 0, 2, 1).astype(bf16)
    )
    # per-core rhs: memT shard [D, N_PAD] with zero padding
    rhs_all = np.zeros((N_CORES, D, N_PAD), bf16)
    rhs_all[:, :, :N_SHARD] = (
        memory.reshape(N_CORES, N_SHARD, D).transpose(0, 2, 1).astype(bf16)
    )
    return lhs_np, rhs_all


def kernel(inputs, targets, memory):
    """Full-input entry point: returns logits [4096, 50000] float32."""
    nc = _build()
    lhs_np, rhs_all = _prep_inputs(inputs, memory)
    in_maps = [{"lhs": lhs_np, "rhs": rhs_all[c]} for c in range(N_CORES)]
    res = run_bass_kernel_spmd(nc, in_maps, core_ids=list(range(N_CORES)))
    logits = np.concatenate(
        [res.results[c]["out"][:, :N_SHARD].astype(np.float32) for c in range(N_CORES)],
        axis=1,
    )
    return np.ascontiguousarray(logits)


# revision 4
# speedup vs baseline: 1.2197x; 1.0157x over previous
"""Trainium2 Bass kernel for nn_MCLoss (scatter_memory forward).

Computes logits = inputs @ memory.T  ([4096, 2048] @ [2048, 50000] -> [4096, 50000] f32).

Strategy (tensor-parallel, per sharding hint): the memory bank is sharded
row-wise across 8 NeuronCores (6250 identity rows each, zero-padded to 6272 =
49*128 columns of the per-core logits slice). Each core computes its
[4096, 6272] slice of the logits with a tiled PE matmul; the host concatenates
the 8 slices and drops the padding.

Device kernel (per core, identical SPMD program), v2:
  - All operands bf16 (PE runs 1 cycle/row, same as fp32r, but half the DMA
    bytes and half the SBUF footprint; rel err ~2e-3 on unit-norm rows, well
    inside the 2e-2 gate).
  - The whole lhs (inputs, pre-transposed on host into [128, 32, 16, 128]
    tile layout: A[p, m, k, j] = inputs[m*128 + j, k*128 + p]) stays RESIDENT
    in SBUF (16 MB = 128 KB/partition), loaded once at kernel start — v1
    re-streamed it once per column group (5x32 MB of DMA), which starved the
    PE. First 4 m-tiles are prefetched before the main loop; the rest stream
    in behind the first column group's compute.
  - rhs (memory shard transposed on host to [2048, 6272]) streams through
    SBUF in 12x512 + 1x128 column groups (one PSUM bank wide), triple
    buffered; each group is reused by all 32 m-tiles.
  - Per (group, m): 16 accumulating matmuls (one per k-tile) into one PSUM
    bank; VectorE evicts PSUM fp32 -> SBUF bf16 (downcast), DMA to the final
    [4096, 6272] bf16 layout; host upcasts to fp32.
  - PE work: 32*16*6272 = 3.21M cycles @ 2.4 GHz ~= 1.34 ms/core; total DMA
    ~93 MB/core (~70 GB/s), far under the ~400 GB/s fabric, so the kernel
    should sit at the PE roofline.
"""
import numpy as np
import ml_dtypes

import concourse.bass as bass
import concourse.mybir as mybir
import concourse.tile as tile
from concourse import bacc
from concourse.bass_utils import run_bass_kernel_spmd

P = 128
B = 4096          # rows of inputs
D = 2048          # features (contraction)
C = 50000         # memory rows (classes)
N_CORES = 8
N_SHARD = C // N_CORES          # 6250, per-core logits width (exact, no pad)
GW = 512                        # column-group width (one PSUM bank of fp32)
MT = B // P                     # 32
KT = D // P                     # 16
LHS_PREFETCH = 4                # m-tiles loaded before the main loop

_NC_CACHE = {}


def _groups():
    # Small remainder group first: its rhs DMA is ~430 KB vs 2 MB, so the PE
    # starts ~3 us earlier; the 12 full 512-wide groups follow.
    n_full = N_SHARD // GW                       # 12
    rem = N_SHARD - n_full * GW                  # 106
    out = [(n_full * GW, rem)] if rem else []
    out += [(i * GW, GW) for i in range(n_full)]
    return out


def _build():
    if "nc" in _NC_CACHE:
        return _NC_CACHE["nc"]
    dt_in = mybir.dt.bfloat16
    nc = bacc.Bacc("TRN2", target_bir_lowering=False, debug=False)
    lhs = nc.dram_tensor("lhs", [P, MT, KT, P], dt_in, kind="ExternalInput")
    rhs = nc.dram_tensor("rhs", [D, N_SHARD], dt_in, kind="ExternalInput")
    out = nc.dram_tensor("out", [B, N_SHARD], dt_in, kind="ExternalOutput")
    rhs_r = rhs[:].rearrange("(k p) c -> p k c", p=P)

    with tile.TileContext(nc) as tc:
        with (
            tc.tile_pool(name="rhsp", bufs=3) as rhsp,
            tc.tile_pool(name="lhsp", bufs=1) as lhsp,
            tc.tile_pool(name="outp", bufs=4) as outp,
            tc.tile_pool(name="psump", bufs=4, space="PSUM") as psump,
        ):
            lhs_tiles = [None] * MT

            def _load_lhs(m):
                lt = lhsp.tile([P, KT, P], dt_in, tag=f"lhs{m}")
                nc.sync.dma_start(out=lt[:], in_=lhs[:, m, :, :])
                lhs_tiles[m] = lt

            for gi, (c0, w) in enumerate(_groups()):
                rt = rhsp.tile([P, KT, w], dt_in, tag="rhs")
                nc.sync.dma_start(out=rt[:], in_=rhs_r[:, :, c0 : c0 + w])
                if gi == 0:
                    for m in range(LHS_PREFETCH):
                        _load_lhs(m)
                for m in range(MT):
                    if gi == 0 and m + LHS_PREFETCH < MT:
                        _load_lhs(m + LHS_PREFETCH)
                    ps = psump.tile([P, w], mybir.dt.float32, tag="ps")
                    for k in range(KT):
                        nc.tensor.matmul(
                            ps[:],
                            lhsT=lhs_tiles[m][:, k, :],
                            rhs=rt[:, k, :],
                            start=(k == 0),
                            stop=(k == KT - 1),
                        )
                    ot = outp.tile([P, w], dt_in, tag="out")
                    nc.vector.tensor_copy(out=ot[:], in_=ps[:])
                    nc.sync.dma_start(
                        out=out[m * P : (m + 1) * P, c0 : c0 + w], in_=ot[:]
                    )
    nc.compile()
    _NC_CACHE["nc"] = nc
    return nc


def _prep_inputs(inputs, memory):
    bf16 = ml_dtypes.bfloat16
    inputs = np.ascontiguousarray(np.asarray(inputs, dtype=np.float32))
    memory = np.asarray(memory, dtype=np.float32)
    # lhs tile layout: A[p, m, k, j] = inputs[m*128 + j, k*128 + p]
    lhs_np = np.ascontiguousarray(
        inputs.reshape(MT, P, KT, P).transpose(3, 0, 2, 1).astype(bf16)
    )
    # per-core rhs: memT shard [D, N_SHARD]
    rhs_all = np.ascontiguousarray(
        memory.reshape(N_CORES, N_SHARD, D).transpose(0, 2, 1).astype(bf16)
    )
    return lhs_np, rhs_all


def kernel(inputs, targets, memory):
    """Full-input entry point: returns logits [4096, 50000] float32."""
    nc = _build()
    lhs_np, rhs_all = _prep_inputs(inputs, memory)
    in_maps = [{"lhs": lhs_np, "rhs": rhs_all[c]} for c in range(N_CORES)]
    res = run_bass_kernel_spmd(nc, in_maps, core_ids=list(range(N_CORES)))
    logits = np.concatenate(
        [res.results[c]["out"].astype(np.float32) for c in range(N_CORES)],
        axis=1,
    )
    return np.ascontiguousarray(logits)


# revision 6
# speedup vs baseline: 1.2369x; 1.0141x over previous
"""Trainium2 Bass kernel for nn_MCLoss (scatter_memory forward).

Computes logits = inputs @ memory.T  ([4096, 2048] @ [2048, 50000] -> [4096, 50000] f32).

Strategy (tensor-parallel, per sharding hint): the memory bank is sharded
row-wise across 8 NeuronCores (6250 identity rows each). Each core computes
its [4096, 6250] slice of the logits with a tiled PE matmul; the host
concatenates the 8 slices.

Device kernel (per core, identical SPMD program), v3:
  - All operands bf16 (PE runs 1 cycle/row, same as fp32r, but half the DMA
    bytes and half the SBUF footprint; rel err ~2.6e-3 on unit-norm rows,
    well inside the 2e-2 gate).
  - The whole lhs (inputs, pre-transposed on host into [128, 32, 16, 128]
    tile layout: A[p, m, k, j] = inputs[m*128 + j, k*128 + p]) stays RESIDENT
    in SBUF (16 MB = 128 KB/partition), loaded once at kernel start — v1
    re-streamed it once per column group (5x32 MB of DMA), which starved the
    PE. First 4 m-tiles are prefetched before the main loop; the rest stream
    in behind the first column group's compute.
  - rhs (memory shard transposed on host to [2048, 6250]) streams through
    SBUF in 12x512 + 1x106 column groups (one PSUM bank wide), triple
    buffered; each group is reused by all 32 m-tiles.
  - Per (group, m): 16 accumulating matmuls (one per k-tile) into one PSUM
    bank; VectorE evicts PSUM fp32 -> SBUF bf16 (downcast), DMA to the final
    [4096, 6250] bf16 layout; host upcasts to fp32.
  - PE work: 32*16*6250 + per-MM overhead ~= 3.24M cycles ~= 1.35 ms/core at
    the 2.4 GHz nominal clock (TimelineSim agrees: 1.356 ms); sustained
    8-core load downclocks the PE to ~2.0 GHz (P0), so ~1.6 ms on HW. Total
    DMA ~93 MB/core (~60 GB/s), far under the ~400 GB/s fabric.
"""
import numpy as np
import ml_dtypes

import concourse.bass as bass
import concourse.mybir as mybir
import concourse.tile as tile
from concourse import bacc
from concourse.bass_utils import run_bass_kernel_spmd

P = 128
B = 4096          # rows of inputs
D = 2048          # features (contraction)
C = 50000         # memory rows (classes)
N_CORES = 8
N_SHARD = C // N_CORES          # 6250, per-core logits width (exact, no pad)
GW = 512                        # column-group width (one PSUM bank of fp32)
MT = B // P                     # 32
KT = D // P                     # 16
LHS_PREFETCH = 4                # m-tiles loaded before the main loop

_NC_CACHE = {}


def _groups():
    n_full = N_SHARD // GW                       # 12
    rem = N_SHARD - n_full * GW                  # 106
    out = [(i * GW, GW) for i in range(n_full)]
    if rem:
        out.append((n_full * GW, rem))
    return out


def _build():
    if "nc" in _NC_CACHE:
        return _NC_CACHE["nc"]
    dt_in = mybir.dt.bfloat16
    nc = bacc.Bacc("TRN2", target_bir_lowering=False, debug=False)
    lhs = nc.dram_tensor("lhs", [P, MT, KT, P], dt_in, kind="ExternalInput")
    rhs = nc.dram_tensor("rhs", [D, N_SHARD], dt_in, kind="ExternalInput")
    out = nc.dram_tensor("out", [B, N_SHARD], dt_in, kind="ExternalOutput")
    rhs_r = rhs[:].rearrange("(k p) c -> p k c", p=P)

    with tile.TileContext(nc) as tc:
        with (
            tc.tile_pool(name="rhsp", bufs=3) as rhsp,
            tc.tile_pool(name="lhsp", bufs=1) as lhsp,
            tc.tile_pool(name="outp", bufs=4) as outp,
            tc.tile_pool(name="psump", bufs=4, space="PSUM") as psump,
        ):
            lhs_tiles = [None] * MT

            def _load_lhs(m):
                lt = lhsp.tile([P, KT, P], dt_in, tag=f"lhs{m}")
                nc.sync.dma_start(out=lt[:], in_=lhs[:, m, :, :])
                lhs_tiles[m] = lt

            for gi, (c0, w) in enumerate(_groups()):
                rt = rhsp.tile([P, KT, w], dt_in, tag="rhs")
                nc.sync.dma_start(out=rt[:], in_=rhs_r[:, :, c0 : c0 + w])
                if gi == 0:
                    for m in range(LHS_PREFETCH):
                        _load_lhs(m)
                for m in range(MT):
                    if gi == 0 and m + LHS_PREFETCH < MT:
                        _load_lhs(m + LHS_PREFETCH)
                    ps = psump.tile([P, w], mybir.dt.float32, tag="ps")
                    for k in range(KT):
                        nc.tensor.matmul(
                            ps[:],
                            lhsT=lhs_tiles[m][:, k, :],
                            rhs=rt[:, k, :],
                            start=(k == 0),
                            stop=(k == KT - 1),
                        )
                    ot = outp.tile([P, w], dt_in, tag="out")
                    nc.vector.tensor_copy(out=ot[:], in_=ps[:])
                    nc.sync.dma_start(
                        out=out[m * P : (m + 1) * P, c0 : c0 + w], in_=ot[:]
                    )
    nc.compile()
    _NC_CACHE["nc"] = nc
    return nc


def _prep_inputs(inputs, memory):
    bf16 = ml_dtypes.bfloat16
    inputs = np.ascontiguousarray(np.asarray(inputs, dtype=np.float32))
    memory = np.asarray(memory, dtype=np.float32)
    # lhs tile layout: A[p, m, k, j] = inputs[m*128 + j, k*128 + p]
    lhs_np = np.ascontiguousarray(
        inputs.reshape(MT, P, KT, P).transpose(3, 0, 2, 1).astype(bf16)
    )
    # per-core rhs: memT shard [D, N_SHARD]
    rhs_all = np.ascontiguousarray(
        memory.reshape(N_CORES, N_SHARD, D).transpose(0, 2, 1).astype(bf16)
    )
    return lhs_np, rhs_all


def kernel(inputs, targets, memory):
    """Full-input entry point: returns logits [4096, 50000] float32."""
    nc = _build()
    lhs_np, rhs_all = _prep_inputs(inputs, memory)
    in_maps = [{"lhs": lhs_np, "rhs": rhs_all[c]} for c in range(N_CORES)]
    res = run_bass_kernel_spmd(nc, in_maps, core_ids=list(range(N_CORES)))
    logits = np.concatenate(
        [res.results[c]["out"].astype(np.float32) for c in range(N_CORES)],
        axis=1,
    )
    return np.ascontiguousarray(logits)


# revision 7
# speedup vs baseline: 1.2771x; 1.0325x over previous
"""Trainium2 Bass kernel for nn_MCLoss (scatter_memory forward).

Computes logits = inputs @ memory.T  ([4096, 2048] @ [2048, 50000] -> [4096, 50000] f32).

Strategy (tensor-parallel, per sharding hint): the memory bank is sharded
row-wise across 8 NeuronCores (6250 identity rows each). Each core computes
its [4096, 6250] slice of the logits with a tiled PE matmul; the host
concatenates the 8 slices.

Device kernel (per core, identical SPMD program), v3:
  - All operands bf16 (PE runs 1 cycle/row, same as fp32r, but half the DMA
    bytes and half the SBUF footprint; rel err ~2.6e-3 on unit-norm rows,
    well inside the 2e-2 gate).
  - The whole lhs (inputs, pre-transposed on host into [128, 32, 16, 128]
    tile layout: A[p, m, k, j] = inputs[m*128 + j, k*128 + p]) stays RESIDENT
    in SBUF (16 MB = 128 KB/partition), loaded once at kernel start — v1
    re-streamed it once per column group (5x32 MB of DMA), which starved the
    PE. First 4 m-tiles are prefetched before the main loop; the rest stream
    in behind the first column group's compute.
  - rhs (memory shard transposed on host to [2048, 6250]) streams through
    SBUF in 12x512 + 1x106 column groups (one PSUM bank wide), triple
    buffered; each group is reused by all 32 m-tiles.
  - Per (group, m): 16 accumulating matmuls (one per k-tile) into one PSUM
    bank; VectorE evicts PSUM fp32 -> SBUF bf16 (downcast), DMA to the final
    [4096, 6250] bf16 layout; host upcasts to fp32.
  - PE work: 32*16*6250 + per-MM overhead ~= 3.24M cycles ~= 1.35 ms/core at
    the 2.4 GHz nominal clock (TimelineSim agrees: 1.356 ms); sustained
    8-core load downclocks the PE to ~2.0 GHz (P0), so ~1.6 ms on HW. Total
    DMA ~93 MB/core (~60 GB/s), far under the ~400 GB/s fabric.
"""
import numpy as np
import ml_dtypes

import concourse.bass as bass
import concourse.mybir as mybir
import concourse.tile as tile
from concourse import bacc
from concourse.bass_utils import run_bass_kernel_spmd

P = 128
B = 4096          # rows of inputs
D = 2048          # features (contraction)
C = 50000         # memory rows (classes)
N_CORES = 8
N_SHARD = C // N_CORES          # 6250, per-core logits width (exact, no pad)
GW = 512                        # column-group width (one PSUM bank of fp32)
MT = B // P                     # 32
KT = D // P                     # 16
LHS_PREFETCH = 4                # m-tiles loaded before the main loop

_NC_CACHE = {}


def _groups():
    n_full = N_SHARD // GW                       # 12
    rem = N_SHARD - n_full * GW                  # 106
    out = [(i * GW, GW) for i in range(n_full)]
    if rem:
        out.append((n_full * GW, rem))
    return out


def _build():
    if "nc" in _NC_CACHE:
        return _NC_CACHE["nc"]
    dt_in = mybir.dt.bfloat16
    nc = bacc.Bacc("TRN2", target_bir_lowering=False, debug=False)
    lhs = nc.dram_tensor("lhs", [P, MT, KT, P], dt_in, kind="ExternalInput")
    rhs = nc.dram_tensor("rhs", [D, N_SHARD], dt_in, kind="ExternalInput")
    out = nc.dram_tensor("out", [B, N_SHARD], dt_in, kind="ExternalOutput")
    rhs_r = rhs[:].rearrange("(k p) c -> p k c", p=P)

    with tile.TileContext(nc) as tc:
        with (
            tc.tile_pool(name="rhsp", bufs=4) as rhsp,
            tc.tile_pool(name="lhsp", bufs=1) as lhsp,
            tc.tile_pool(name="outp", bufs=6) as outp,
            tc.tile_pool(name="psump", bufs=6, space="PSUM") as psump,
        ):
            lhs_tiles = [None] * MT

            def _load_lhs(m):
                lt = lhsp.tile([P, KT, P], dt_in, tag=f"lhs{m}")
                nc.sync.dma_start(out=lt[:], in_=lhs[:, m, :, :])
                lhs_tiles[m] = lt

            for gi, (c0, w) in enumerate(_groups()):
                rt = rhsp.tile([P, KT, w], dt_in, tag="rhs")
                nc.sync.dma_start(out=rt[:], in_=rhs_r[:, :, c0 : c0 + w])
                if gi == 0:
                    for m in range(LHS_PREFETCH):
                        _load_lhs(m)
                for m in range(MT):
                    if gi == 0 and m + LHS_PREFETCH < MT:
                        _load_lhs(m + LHS_PREFETCH)
                    ps = psump.tile([P, w], mybir.dt.float32, tag="ps")
                    for k in range(KT):
                        nc.tensor.matmul(
                            ps[:],
                            lhsT=lhs_tiles[m][:, k, :],
                            rhs=rt[:, k, :],
                            start=(k == 0),
                            stop=(k == KT - 1),
                        )
                    ot = outp.tile([P, w], dt_in, tag="out")
                    nc.vector.tensor_copy(out=ot[:], in_=ps[:])
                    nc.sync.dma_start(
                        out=out[m * P : (m + 1) * P, c0 : c0 + w], in_=ot[:]
                    )
    nc.compile()
    _NC_CACHE["nc"] = nc
    return nc


def _prep_inputs(inputs, memory):
    bf16 = ml_dtypes.bfloat16
    inputs = np.ascontiguousarray(np.asarray(inputs, dtype=np.float32))
    memory = np.asarray(memory, dtype=np.float32)
    # lhs tile layout: A[p, m, k, j] = inputs[m*128 + j, k*128 + p]
    lhs_np = np.ascontiguousarray(
        inputs.reshape(MT, P, KT, P).transpose(3, 0, 2, 1).astype(bf16)
    )
    # per-core rhs: memT shard [D, N_SHARD]
    rhs_all = np.ascontiguousarray(
        memory.reshape(N_CORES, N_SHARD, D).transpose(0, 2, 1).astype(bf16)
    )
    return lhs_np, rhs_all


def kernel(inputs, targets, memory):
    """Full-input entry point: returns logits [4096, 50000] float32."""
    nc = _build()
    lhs_np, rhs_all = _prep_inputs(inputs, memory)
    in_maps = [{"lhs": lhs_np, "rhs": rhs_all[c]} for c in range(N_CORES)]
    res = run_bass_kernel_spmd(nc, in_maps, core_ids=list(range(N_CORES)))
    logits = np.concatenate(
        [res.results[c]["out"].astype(np.float32) for c in range(N_CORES)],
        axis=1,
    )
    return np.ascontiguousarray(logits)
